# revision 12
# baseline (speedup 1.0000x reference)
"""Multi-head causal attention + residual + RMSNorm, 8-core Trainium2 Bass kernel.

Sharding: core c = (batch b = c//2, group g = c%2). Group g owns the 8
query blocks {i : i % 2 == g} of the 16 x 128-row blocks of T=2048.
Each core computes full K/V projections for its batch, Q projection for
its packed 1024 query rows, causal attention (all 16 heads), the wo
projection, residual add and RMSNorm for its rows. No collectives; the
host only slices inputs and concatenates outputs.

The program is SPMD-uniform: per-core causality differences enter only
through per-core inputs (xT/xres row selection and a mask tensor:
[tri, zero] for even groups, [ones, tri] for odd groups).

Numerics/layout: all projection and attention-value matmuls run in fp8
e4m3 with the PE DoubleRow perf mode (two 128-row k-tiles per pass, 2x
column throughput). Weights arrive from the host pre-packed in the
DoubleRow pair layout (one stacked fp8 tensor); x is converted to fp8
pairs on-chip. Scores stay bf16 (64-deep contraction cannot pair
k-tiles). Scores are computed transposed (scoresT[k,q] = kT.T @ qT) so
exp(scoresT) (fp8 out, straight from the Activation engine) feeds the AV
matmul as the moving operand with token-major fp8 V pairs as the
stationary operand -- no transposes anywhere. A ones column appended to
each V tile makes the softmax denominator appear as PSUM row 64 for
free. Each head is normalized at PSUM drain by broadcasting
1/denominator across the 64 hd partitions via a DRAM-roundtrip DMA.

Scheduling: PSUM drains and fp8 conversions rotate across the DVE,
Activation and GpSimd engines so no single engine serializes the
projection phase; the attention loop is software-pipelined one (head,
key-pair) step ahead so the next step's scores+exp are emitted before
the current step's AV matmuls, keeping the Activation engine (the
bottleneck) saturated across head boundaries. All f32 host tensors ride
in one flat DRAM blob and all fp8 weights in a second one, minimizing
the per-launch buffer-handle count on the host dispatch path.
"""

import math
import os
from contextlib import ExitStack

import numpy as np
import ml_dtypes

import concourse.bass as bass
import concourse.bacc as bacc
import concourse.tile as tile
from concourse import mybir

B, T, D, H, HD = 4, 2048, 1024, 16, 64
P = 128
NB = T // P          # 16 key blocks
QB = NB // 2         # 8 query blocks per core
NQ = QB * P          # 1024 query rows per core
DC = D // P          # 8 chunks of the model dim
DP = DC // 2         # 4 DoubleRow chunk pairs
VW = HD + 1          # V tile width per head (ones column appended)
EPS = 1e-6
BF = mybir.dt.bfloat16
F32 = mybir.dt.float32
F8 = mybir.dt.float8e4
NPF8 = ml_dtypes.float8_e4m3
FP = mybir.ActivationFunctionType
OP = mybir.AluOpType
DR = mybir.MatmulPerfMode.DoubleRow

# flat f32 blob layout: xT rows (D x T, token-permuted: the core's 8
# query blocks first, then the other 8), xres, mask, norm_g
XW = T
XRES_OFF = D * XW
MSK_OFF = XRES_OFF + NQ * D
G_OFF = MSK_OFF + 2 * P * P
XBIG_LEN = G_OFF + D

TRACE = False
LAST_RESULTS = None
LAST_IN_MAPS = None
_NC_CACHE = {}


def _copy(eng, out, in_):
    if hasattr(eng, "tensor_copy"):
        eng.tensor_copy(out=out, in_=in_)
    else:
        eng.copy(out=out, in_=in_)


def _av_segments(a, b):
    while a < b:
        e = min(b, (a // 512 + 1) * 512)
        yield a, e
        a = e


def build_nc():
    nc = bacc.Bacc("TRN2", target_bir_lowering=False, debug=False, num_devices=8)

    xbig = nc.dram_tensor("xbig", [XBIG_LEN], F32, kind="ExternalInput").ap()
    wall = nc.dram_tensor("wall", [4, DP, P, 2, D], F8, kind="ExternalInput").ap()
    yout = nc.dram_tensor("y", [NQ, D], BF, kind="ExternalOutput").ap()

    def xv(offset, ap):
        return bass.AP(tensor=xbig.tensor, offset=xbig.offset + offset, ap=ap)

    xT_s = lambda d, hf: xv(d * P * XW + hf * 1024, [[XW, P], [1, 1024]])
    xres_s = lambda j: xv(XRES_OFF + j * P * D, [[D, P], [1, D]])
    msk_s = xv(MSK_OFF, [[P, P], [P * P, 2], [1, P]])
    g_s = xv(G_OFF, [[0, P], [1, D]])

    with tile.TileContext(nc) as tc, ExitStack() as top:
        rlong = top.enter_context(tc.tile_pool(name="rlong", bufs=1))
        stg = top.enter_context(tc.tile_pool(name="stg", bufs=6))
        dpool = top.enter_context(tc.tile_pool(name="dram", bufs=1, space="DRAM"))

        # long-lived tiles
        aT_f8 = [rlong.tile([P, 2 * NQ], F8, tag=f"aT{c}", name=f"aT{c}")
                 .rearrange("p (t q) -> p t q", t=2) for c in range(DP)]
        wo_f8 = [rlong.tile([P, 2 * D], F8, tag=f"wo{c}", name=f"wo{c}")
                 .rearrange("p (t j) -> p t j", t=2) for c in range(DP)]
        g_sb = rlong.tile([P, D], F32, tag="g")
        mask_f8 = rlong.tile([P, 2 * P], F8, tag="mask")
        eps_sb = rlong.tile([P, 1], F32, tag="eps")
        xr_sb = [rlong.tile([P, D], F32, tag=f"xr{j}", name=f"xr{j}")
                 for j in range(QB)]
        nc.vector.memset(eps_sb, EPS)

        # masks: fp32 -> fp8 [128, 2*128]
        mstage = stg.tile([P, 1024], F32, tag="stg", name="mstage")
        nc.sync.dma_start(out=mstage[:, 0:2 * P].rearrange("p (i q) -> p i q", i=2),
                          in_=msk_s)
        nc.vector.tensor_copy(out=mask_f8, in_=mstage[:, 0:2 * P])
        maskv = mask_f8.rearrange("p (i q) -> p i q", i=2)

        with tc.tile_pool(name="rmid", bufs=1) as rmid:
            xp = [rmid.tile([P, 2 * T], F8, tag=f"xp{d}", name=f"xp{d}")
                  .rearrange("p (t x) -> p t x", t=2) for d in range(DP)]
            kT_sb = [rmid.tile([P, T], BF, tag=f"kT{c}", name=f"kT{c}")
                     for c in range(DC)]
            qT_sb = [rmid.tile([P, NQ], BF, tag=f"qT{c}", name=f"qT{c}")
                     for c in range(DC)]
            vp = [rmid.tile([P, 2 * H * VW], F8, tag=f"v{t}", name=f"v{t}")
                  .rearrange("p (t h e) -> p t h e", t=2, h=H) for t in range(QB)]

            ROT = [None, None, None]

            def rot(i):
                return (nc.vector, nc.scalar, nc.gpsimd)[i % 3]

            def rot2(i):
                # PSUM-reading drains: GpSimd has no PSUM port
                return (nc.vector, nc.scalar)[i % 2]

            # ---------------- Phase AB: projections ----------------
            with tc.tile_pool(name="pa", bufs=1) as pa, \
                 tc.tile_pool(name="psA", bufs=3, space="PSUM") as psA:
                wq_f8 = [pa.tile([P, 2 * D], F8, tag=f"wq{d}", name=f"wq{d}")
                         .rearrange("p (t j) -> p t j", t=2) for d in range(DP)]
                wk_f8 = [pa.tile([P, 2 * D], F8, tag=f"wk{d}", name=f"wk{d}")
                         .rearrange("p (t j) -> p t j", t=2) for d in range(DP)]
                wv_f8 = [pa.tile([P, 2 * D], F8, tag=f"wv{d}", name=f"wv{d}")
                         .rearrange("p (t j) -> p t j", t=2) for d in range(DP)]
                for dp in range(DP):
                    nc.sync.dma_start(out=wq_f8[dp], in_=wall[0][dp])
                # x -> fp8 pair layout (converts rotate DVE/ACT/GpSimd);
                # the permuted layout puts the core's query blocks in the
                # first half, so Q can start once hf=0 is converted
                for hf in range(2):
                    for d in range(DC):
                        s = stg.tile([P, 1024], F32, tag="stg", name="sx")
                        nc.sync.dma_start(out=s, in_=xT_s(d, hf))
                        _copy(rot(hf * DC + d),
                              xp[d // 2][:, d % 2, hf * 1024:(hf + 1) * 1024],
                              s)
                    if hf == 0:
                        for dp in range(DP):
                            nc.sync.dma_start(out=wk_f8[dp], in_=wall[1][dp])
                            nc.sync.dma_start(out=wv_f8[dp], in_=wall[2][dp])
                # Q projection
                for c in range(DC):
                    pt = psA.tile([P, NQ], F32, tag="psA")
                    for dp in range(DP):
                        for off in range(0, NQ, 512):
                            nc.tensor.matmul(
                                pt[:, off:off + 512],
                                lhsT=wq_f8[dp][:, :, c * P:(c + 1) * P],
                                rhs=xp[dp][:, :, off:off + 512],
                                start=(dp == 0), stop=(dp == DP - 1),
                                perf_mode=DR)
                    _copy(rot2(c), qT_sb[c], pt)
                # K projection
                for c in range(DC):
                    for hf in range(2):
                        pt = psA.tile([P, 1024], F32, tag="psA")
                        for dp in range(DP):
                            for off in range(0, 1024, 512):
                                nc.tensor.matmul(
                                    pt[:, off:off + 512],
                                    lhsT=wk_f8[dp][:, :, c * P:(c + 1) * P],
                                    rhs=xp[dp][:, :, hf * 1024 + off:
                                               hf * 1024 + off + 512],
                                    start=(dp == 0), stop=(dp == DP - 1),
                                    perf_mode=DR)
                        _copy(rot2(c * 2 + hf),
                              kT_sb[c][:, hf * 1024:(hf + 1) * 1024], pt)
                # V (token-major) with ones column per head
                for t in range(NB):
                    pt = psA.tile([P, D], F32, tag="psA")
                    for dp in range(DP):
                        for off in range(0, D, 512):
                            nc.tensor.matmul(
                                pt[:, off:off + 512],
                                lhsT=xp[dp][:, :, t * P:(t + 1) * P],
                                rhs=wv_f8[dp][:, :, off:off + 512],
                                start=(dp == 0), stop=(dp == DP - 1),
                                perf_mode=DR)
                    vv = vp[t % QB]
                    _copy(rot2(t), vv[:, t // QB, :, 0:HD],
                          pt.rearrange("p (h e) -> p h e", h=H))
                    nc.gpsimd.memset(vv[:, t // QB, :, HD:HD + 1], 1.0)

            # phase-D weights/params + residual rows prefetch while
            # attention runs
            for cp in range(DP):
                nc.sync.dma_start(out=wo_f8[cp], in_=wall[3][cp])
            nc.gpsimd.dma_start(out=g_sb, in_=g_s)
            for j in range(QB):
                nc.sync.dma_start(out=xr_sb[j], in_=xres_s(j))

            # ---------------- Phase C: attention ----------------
            # Software-pipelined: step (h, jp) emits scores+exp; the
            # previous step's mask+AV (+ head normalize) trail one step
            # behind so the next scores are already in flight when the
            # Activation engine finishes the current exp.
            with tc.tile_pool(name="pexp", bufs=6) as pexp, \
                 tc.tile_pool(name="prec", bufs=2) as prec, \
                 tc.tile_pool(name="psS", bufs=2, space="PSUM") as psS, \
                 tc.tile_pool(name="psO", bufs=2, space="PSUM") as psO:
                po_by_h = {}
                pending = None

                def emit_tail(h, jp, et):
                    po = po_by_h[h]
                    p0 = jp * P
                    eng = nc.vector if (h * QB + jp) % 2 == 0 else nc.gpsimd
                    eng.tensor_mul(et[:, :, 0:P], et[:, :, 0:P], maskv)
                    lw = vp[jp][:, :, h, :]
                    for sa, se in _av_segments(p0, NQ):
                        # stop only on the terminal write of each PSUM
                        # bank (group tracking is per 2KB zero-region)
                        nc.tensor.matmul(
                            po[0:VW, sa:se],
                            lhsT=lw, rhs=et[:, :, sa - p0:se - p0],
                            start=(jp == 0),
                            stop=(jp % 4 == 3 and sa == p0),
                            perf_mode=DR)
                    if jp == QB - 1:
                        # normalize head h: broadcast 1/den across the 64
                        # hd partitions via a DRAM roundtrip
                        ch, r0 = h // 2, (h % 2) * HD
                        rec = prec.tile([1, NQ], F32, tag="rec", name="rec")
                        nc.vector.reciprocal(rec, po[HD:HD + 1, :])
                        rec_d = dpool.tile([NQ], F32, tag="rec_d",
                                           name="rec_d", bufs=2)
                        nc.sync.dma_start(out=rec_d, in_=rec)
                        rb = prec.tile([HD, NQ], F32, tag="rb", name="rb")
                        rb_bc = bass.AP(tensor=rec_d.tensor,
                                        offset=rec_d.offset,
                                        ap=[[0, HD], list(rec_d.ap[0])])
                        nc.sync.dma_start(out=rb, in_=rb_bc)
                        nc.vector.tensor_mul(
                            aT_f8[ch // 2][r0:r0 + HD, ch % 2, :],
                            po[0:HD, :], rb)
                        del po_by_h[h]

                for h in range(H):
                    ch, r0 = h // 2, (h % 2) * HD
                    po_by_h[h] = psO.tile([P, NQ], F32, tag="psO", name="po")
                    for jp in range(QB):
                        p0 = jp * P
                        ntail = NQ - p0
                        et = pexp.tile([P, 2 * NQ], F8, tag="expT", name="et") \
                            .rearrange("p (t q) -> p t q", t=2)
                        a = 0
                        while a < ntail:
                            w = min(512, ntail - a)
                            ps2 = psS.tile([P, 2 * 512], F32, tag="psS",
                                           name="ps2") \
                                .rearrange("p (t q) -> p t q", t=2)
                            for t in range(2):
                                kb = jp + QB * t
                                nc.tensor.matmul(
                                    ps2[:, t, 0:w],
                                    lhsT=kT_sb[ch][r0:r0 + HD,
                                                   kb * P:(kb + 1) * P],
                                    rhs=qT_sb[ch][r0:r0 + HD,
                                                  p0 + a:p0 + a + w],
                                    start=True, stop=True)
                            nc.scalar.activation(out=et[:, :, a:a + w],
                                                 in_=ps2[:, :, 0:w],
                                                 func=FP.Exp, scale=0.125)
                            a += w
                        if pending is not None:
                            emit_tail(*pending)
                        pending = (h, jp, et)
                if pending is not None:
                    emit_tail(*pending)

        # ---------------- Phase D: wo + residual + RMSNorm ----------------
        with tc.tile_pool(name="py", bufs=3) as pyp, \
             tc.tile_pool(name="psY", bufs=2, space="PSUM") as psY:
            for j in range(QB):
                py = psY.tile([P, D], F32, tag="psY")
                for cp in range(DP):
                    for off in range(0, D, 512):
                        nc.tensor.matmul(
                            py[:, off:off + 512],
                            lhsT=aT_f8[cp][:, :, j * P:(j + 1) * P],
                            rhs=wo_f8[cp][:, :, off:off + 512],
                            start=(cp == 0), stop=(cp == DP - 1),
                            perf_mode=DR)
                ysb = pyp.tile([P, D], F32, tag="ysb")
                nc.vector.tensor_add(ysb, py, xr_sb[j])
                sq = pyp.tile([P, D], F32, tag="sq")
                ss = pyp.tile([P, 1], F32, tag="ss")
                nc.scalar.activation(out=sq, in_=ysb, func=FP.Square,
                                     accum_out=ss)
                rstd = pyp.tile([P, 1], F32, tag="rstd")
                nc.scalar.activation(out=rstd, in_=ss, func=FP.Sqrt,
                                     scale=1.0 / D, bias=eps_sb)
                nc.vector.reciprocal(rstd, rstd)
                osb = pyp.tile([P, D], BF, tag="osb")
                nc.vector.scalar_tensor_tensor(
                    out=osb, in0=ysb, scalar=rstd, in1=g_sb,
                    op0=OP.mult, op1=OP.mult)
                nc.sync.dma_start(out=yout[j * P:(j + 1) * P, :], in_=osb)

    nc.compile()
    return nc


N_CORES = 8


def _make_runner(nc):
    import jax
    from jax.experimental.shard_map import shard_map
    from jax.sharding import Mesh, NamedSharding, PartitionSpec
    from concourse import bass2jax

    bass2jax.install_neuronx_cc_hook()
    partition_name = (nc.partition_id_tensor.name
                      if nc.partition_id_tensor else None)
    in_names, out_names, out_avals = [], [], []
    for alloc in nc.m.functions[0].allocations:
        if not isinstance(alloc, mybir.MemoryLocationSet):
            continue
        name = alloc.memorylocations[0].name
        if alloc.kind == "ExternalInput":
            if name != partition_name:
                in_names.append(name)
        elif alloc.kind == "ExternalOutput":
            out_names.append(name)
            out_avals.append(jax.core.ShapedArray(
                tuple(alloc.tensor_shape), mybir.dt.np(alloc.dtype)))
    n_params = len(in_names)
    # No output-seed operands: the kernel writes every element of y, so
    # the custom call's uninitialized result buffer is fully overwritten.
    all_in = list(in_names)
    if partition_name is not None:
        all_in.append(partition_name)

    def _body(*args):
        operands = list(args)
        if partition_name is not None:
            operands.append(bass2jax.partition_id_tensor())
        outs = bass2jax._bass_exec_p.bind(
            *operands,
            out_avals=tuple(out_avals),
            in_names=tuple(all_in),
            out_names=tuple(out_names),
            lowering_input_output_aliases=(),
            sim_require_finite=True,
            sim_require_nnan=True,
            nc=nc,
        )
        return tuple(outs)

    devices = jax.devices()[:N_CORES]
    mesh = Mesh(np.asarray(devices), ("core",))
    smapped = shard_map(_body, mesh=mesh,
                        in_specs=(PartitionSpec("core"),) * n_params,
                        out_specs=(PartitionSpec("core"),) * len(out_names),
                        check_rep=False)
    # Fast-dispatch (effect-free) compile keeps the per-launch client
    # overhead low so deep pipelines of in-flight executes stay fed.
    sh = NamedSharding(mesh, PartitionSpec("core"))
    in_sds = []
    for alloc in nc.m.functions[0].allocations:
        if not isinstance(alloc, mybir.MemoryLocationSet):
            continue
        name = alloc.memorylocations[0].name
        if alloc.kind == "ExternalInput" and name != partition_name:
            in_sds.append(jax.ShapeDtypeStruct(
                (N_CORES * alloc.tensor_shape[0], *alloc.tensor_shape[1:]),
                mybir.dt.np(alloc.dtype), sharding=sh))
    sharded = bass2jax.fast_dispatch_compile(
        lambda: jax.jit(smapped, keep_unused=True).lower(*in_sds).compile())
    return {"fn": sharded, "in_names": in_names, "out_names": out_names,
            "out_avals": out_avals, "mesh": mesh}


def _get_runner():
    if "runner" not in _NC_CACHE:
        if "nc" not in _NC_CACHE:
            _NC_CACHE["nc"] = build_nc()
        _NC_CACHE["runner"] = _make_runner(_NC_CACHE["nc"])
    return _NC_CACHE["runner"]


def _concat_inputs(r, in_maps):
    return [np.concatenate([np.asarray(in_maps[c][nm]) for c in range(N_CORES)],
                           axis=0)
            for nm in r["in_names"]]


def _run(in_maps):
    r = _get_runner()
    out_arrs = r["fn"](*_concat_inputs(r, in_maps))
    return [
        {nm: np.asarray(out_arrs[i]).reshape(N_CORES, *r["out_avals"][i].shape)[c]
         for i, nm in enumerate(r["out_names"])}
        for c in range(N_CORES)
    ]


def bench(in_maps, iters=3, depth=2048):
    """Per-launch steady-state time of the sharded NEFF execution.

    All inputs are device resident. Each rep launches `depth` executions
    back-to-back without blocking, then blocks once; the amortized
    total/depth is the per-launch service time with the axon-tunnel
    round-trip latency amortized away. Returns one amortized per-launch
    time (seconds) per rep.
    """
    import time
    import jax
    from jax.sharding import NamedSharding, PartitionSpec

    r = _get_runner()
    sh = NamedSharding(r["mesh"], PartitionSpec("core"))
    dev_in = [jax.device_put(a, sh) for a in _concat_inputs(r, in_maps)]
    jax.block_until_ready(dev_in)
    out = r["fn"](*dev_in)
    jax.block_until_ready(out)
    times = []
    for _ in range(iters):
        t0 = time.perf_counter()
        outs = [r["fn"](*dev_in) for _ in range(depth)]
        jax.block_until_ready(outs)
        times.append((time.perf_counter() - t0) / depth)
        del outs
    return times


def _rows(g):
    return np.arange(T).reshape(NB, P)[g::2].ravel()


def _pack_pairs(wT):
    """[D, D] f32 (rows = contraction dim) -> [DP, 128, 2, D] fp8 e4m3."""
    return np.ascontiguousarray(
        wT.reshape(DP, 2, P, D).transpose(0, 2, 1, 3)).astype(NPF8)


def kernel(**inputs):
    global LAST_RESULTS
    x = np.ascontiguousarray(np.asarray(inputs["x"], dtype=np.float32))
    wq = np.asarray(inputs["wq"], dtype=np.float32)
    wk = np.asarray(inputs["wk"], dtype=np.float32)
    wv = np.asarray(inputs["wv"], dtype=np.float32)
    wo = np.asarray(inputs["wo"], dtype=np.float32)
    g = np.ascontiguousarray(np.asarray(inputs["norm_g"], dtype=np.float32))

    if "nc" not in _NC_CACHE:
        _NC_CACHE["nc"] = build_nc()

    wallv = np.stack([_pack_pairs(wq.T), _pack_pairs(wk.T),
                      _pack_pairs(wv.T), _pack_pairs(wo.T)])
    # token perm puts the core's query blocks first, so slot 0 of every
    # key pair is the diagonal block (tri mask) and slot 1 is the other
    # block: fully masked for even groups, fully allowed for odd ones.
    tri = np.triu(np.ones((P, P), np.float32))  # allowed where k <= q
    masks = [np.stack([tri, np.zeros((P, P), np.float32)]),
             np.stack([tri, np.ones((P, P), np.float32)])]

    in_maps = []
    rows_g = [_rows(0), _rows(1)]
    for core in range(8):
        b, gidx = core // 2, core % 2
        rows = rows_g[gidx]
        perm = np.concatenate([rows, rows_g[1 - gidx]])
        xbig = np.concatenate([
            np.ascontiguousarray(x[b].T[:, perm]).ravel(),
            x[b][rows].ravel(),
            masks[gidx].ravel(),
            g,
        ])
        in_maps.append({"xbig": xbig, "wall": wallv})

    global LAST_IN_MAPS
    LAST_IN_MAPS = in_maps
    outs = _run(in_maps)

    y = np.empty((B, T, D), np.float32)
    for core in range(8):
        b, gidx = core // 2, core % 2
        y[b][rows_g[gidx]] = outs[core]["y"].astype(np.float32)
    return y


if __name__ == "__main__":
    rng = np.random.default_rng(0)
    ins = {
        "x": rng.standard_normal((B, T, D), dtype=np.float32),
        "wq": rng.standard_normal((D, D), dtype=np.float32) * 0.02,
        "wk": rng.standard_normal((D, D), dtype=np.float32) * 0.02,
        "wv": rng.standard_normal((D, D), dtype=np.float32) * 0.02,
        "wo": rng.standard_normal((D, D), dtype=np.float32) * 0.02,
        "norm_g": np.ones((D,), np.float32),
    }
    out = kernel(**ins)
    print("ok", out.shape, out.dtype)


# revision 22
# speedup vs baseline: 1.0894x; 1.0894x over previous
"""Multi-head causal attention + residual + RMSNorm, 8-core Trainium2 Bass kernel.

Sharding: core c = (batch b = c//2, group g = c%2). Group g owns the 8
query blocks {i : i % 2 == g} of the 16 x 128-row blocks of T=2048.
Each core computes full K/V projections for its batch, Q projection for
its packed 1024 query rows, causal attention (all 16 heads), the wo
projection, residual add and RMSNorm for its rows. No collectives; the
host only slices inputs and concatenates outputs.

The program is SPMD-uniform: per-core causality differences enter only
through per-core inputs (xT/xres row selection and a mask tensor:
[tri, zero] for even groups, [ones, tri] for odd groups).

Numerics/layout: all projection and attention-value matmuls run in fp8
e4m3 with the PE DoubleRow perf mode (two 128-row k-tiles per pass, 2x
column throughput). Weights arrive from the host pre-packed in the
DoubleRow pair layout (one stacked fp8 tensor); x is converted to fp8
pairs on-chip. Scores stay bf16 (64-deep contraction cannot pair
k-tiles). Scores are computed transposed (scoresT[k,q] = kT.T @ qT) so
exp(scoresT) (fp8 out, straight from the Activation engine) feeds the AV
matmul as the moving operand with token-major fp8 V pairs as the
stationary operand -- no transposes anywhere. A ones column appended to
each V tile makes the softmax denominator appear as PSUM row 64 for
free. Each head is normalized at PSUM drain by broadcasting
1/denominator across the 64 hd partitions via a DRAM-roundtrip DMA.

Scheduling: PSUM drains and fp8 conversions rotate across the DVE,
Activation and GpSimd engines so no single engine serializes the
projection phase; the attention loop is software-pipelined one (head,
key-pair) step ahead so the next step's scores+exp are emitted before
the current step's AV matmuls, keeping the Activation engine (the
bottleneck) saturated across head boundaries. All f32 host tensors ride
in one flat DRAM blob and all fp8 weights in a second one, minimizing
the per-launch buffer-handle count on the host dispatch path.
"""

import math
import os
from contextlib import ExitStack

import numpy as np
import ml_dtypes

import concourse.bass as bass
import concourse.bacc as bacc
import concourse.tile as tile
from concourse import mybir

B, T, D, H, HD = 4, 2048, 1024, 16, 64
P = 128
NB = T // P          # 16 key blocks
QB = NB // 2         # 8 query blocks per core
NQ = QB * P          # 1024 query rows per core
DC = D // P          # 8 chunks of the model dim
DP = DC // 2         # 4 DoubleRow chunk pairs
VW = HD + 1          # V tile width per head (ones column appended)
EPS = 1e-6
BF = mybir.dt.bfloat16
F32 = mybir.dt.float32
F8 = mybir.dt.float8e4
NPF8 = ml_dtypes.float8_e4m3
FP = mybir.ActivationFunctionType
OP = mybir.AluOpType
DR = mybir.MatmulPerfMode.DoubleRow

# flat f32 blob layout: xT rows (D x T, token-permuted: the core's 8
# query blocks first, then the other 8), xres, mask, norm_g
XW = T
XRES_OFF = D * XW
MSK_OFF = XRES_OFF + NQ * D
G_OFF = MSK_OFF + 2 * P * P
XBIG_LEN = G_OFF + D

TRACE = False
LAST_RESULTS = None
LAST_IN_MAPS = None
_NC_CACHE = {}


def _copy(eng, out, in_):
    if hasattr(eng, "tensor_copy"):
        eng.tensor_copy(out=out, in_=in_)
    else:
        eng.copy(out=out, in_=in_)


def _av_segments(a, b):
    while a < b:
        e = min(b, (a // 512 + 1) * 512)
        yield a, e
        a = e


def build_nc():
    nc = bacc.Bacc("TRN2", target_bir_lowering=False, debug=False, num_devices=8)

    xbig = nc.dram_tensor("xbig", [XBIG_LEN], F32, kind="ExternalInput").ap()
    wall = nc.dram_tensor("wall", [4, DP, P, 2, D], F8, kind="ExternalInput").ap()
    yout = nc.dram_tensor("y", [NQ, D], BF, kind="ExternalOutput").ap()

    def xv(offset, ap):
        return bass.AP(tensor=xbig.tensor, offset=xbig.offset + offset, ap=ap)

    xT_s = lambda d, hf: xv(d * P * XW + hf * 1024, [[XW, P], [1, 1024]])
    xres_s = lambda j: xv(XRES_OFF + j * P * D, [[D, P], [1, D]])
    msk_s = xv(MSK_OFF, [[P, P], [P * P, 2], [1, P]])
    g_s = xv(G_OFF, [[0, P], [1, D]])

    with tile.TileContext(nc) as tc, ExitStack() as top:
        rlong = top.enter_context(tc.tile_pool(name="rlong", bufs=1))
        stg = top.enter_context(tc.tile_pool(name="stg", bufs=6))
        dpool = top.enter_context(tc.tile_pool(name="dram", bufs=1, space="DRAM"))

        # long-lived tiles
        aT_f8 = [rlong.tile([P, 2 * NQ], F8, tag=f"aT{c}", name=f"aT{c}")
                 .rearrange("p (t q) -> p t q", t=2) for c in range(DP)]
        wo_f8 = [rlong.tile([P, 2 * D], F8, tag=f"wo{c}", name=f"wo{c}")
                 .rearrange("p (t j) -> p t j", t=2) for c in range(DP)]
        g_sb = rlong.tile([P, D], F32, tag="g")
        mask_f8 = rlong.tile([P, 2 * P], F8, tag="mask")
        eps_sb = rlong.tile([P, 1], F32, tag="eps")
        xr_sb = [rlong.tile([P, D], F32, tag=f"xr{j}", name=f"xr{j}")
                 for j in range(QB)]
        nc.vector.memset(eps_sb, EPS)

        # masks: fp32 -> fp8 [128, 2*128]
        mstage = stg.tile([P, 1024], F32, tag="stg", name="mstage")
        nc.sync.dma_start(out=mstage[:, 0:2 * P].rearrange("p (i q) -> p i q", i=2),
                          in_=msk_s)
        nc.vector.tensor_copy(out=mask_f8, in_=mstage[:, 0:2 * P])
        maskv = mask_f8.rearrange("p (i q) -> p i q", i=2)

        with tc.tile_pool(name="rmid", bufs=1) as rmid:
            xp = [rmid.tile([P, 2 * T], F8, tag=f"xp{d}", name=f"xp{d}")
                  .rearrange("p (t x) -> p t x", t=2) for d in range(DP)]
            kT_sb = [rmid.tile([P, T], BF, tag=f"kT{c}", name=f"kT{c}")
                     for c in range(DC)]
            qT_sb = [rmid.tile([P, NQ], BF, tag=f"qT{c}", name=f"qT{c}")
                     for c in range(DC)]
            vp = [rmid.tile([P, 2 * H * VW], F8, tag=f"v{t}", name=f"v{t}")
                  .rearrange("p (t h e) -> p t h e", t=2, h=H) for t in range(QB)]

            ROT = [None, None, None]

            def rot(i):
                return (nc.vector, nc.scalar, nc.gpsimd)[i % 3]

            def rot2(i):
                # PSUM-reading drains: GpSimd has no PSUM port
                return (nc.vector, nc.scalar)[i % 2]

            # ---------------- Phase AB: projections ----------------
            with tc.tile_pool(name="pa", bufs=1) as pa, \
                 tc.tile_pool(name="psA", bufs=4, space="PSUM") as psA:
                wq_f8 = [pa.tile([P, 2 * D], F8, tag=f"wq{d}", name=f"wq{d}")
                         .rearrange("p (t j) -> p t j", t=2) for d in range(DP)]
                wk_f8 = [pa.tile([P, 2 * D], F8, tag=f"wk{d}", name=f"wk{d}")
                         .rearrange("p (t j) -> p t j", t=2) for d in range(DP)]
                wv_f8 = [pa.tile([P, 2 * D], F8, tag=f"wv{d}", name=f"wv{d}")
                         .rearrange("p (t j) -> p t j", t=2) for d in range(DP)]
                for dp in range(DP):
                    nc.sync.dma_start(out=wq_f8[dp], in_=wall[0][dp])
                # x -> fp8 pair layout (converts rotate DVE/ACT/GpSimd);
                # the permuted layout puts the core's query blocks in the
                # first half, so Q can start once hf=0 is converted
                for hf in range(2):
                    for d in range(DC):
                        s = stg.tile([P, 1024], F32, tag="stg", name="sx")
                        nc.sync.dma_start(out=s, in_=xT_s(d, hf))
                        _copy(rot(hf * DC + d),
                              xp[d // 2][:, d % 2, hf * 1024:(hf + 1) * 1024],
                              s)
                    if hf == 0:
                        for dp in range(DP):
                            nc.sync.dma_start(out=wk_f8[dp], in_=wall[1][dp])
                            nc.sync.dma_start(out=wv_f8[dp], in_=wall[2][dp])
                # Q projection
                for c in range(DC):
                    pt = psA.tile([P, NQ], F32, tag="psA")
                    for dp in range(DP):
                        for off in range(0, NQ, 512):
                            nc.tensor.matmul(
                                pt[:, off:off + 512],
                                lhsT=wq_f8[dp][:, :, c * P:(c + 1) * P],
                                rhs=xp[dp][:, :, off:off + 512],
                                start=(dp == 0), stop=(dp == DP - 1),
                                perf_mode=DR)
                    _copy(rot2(c), qT_sb[c], pt)
                # K projection
                for c in range(DC):
                    for hf in range(2):
                        pt = psA.tile([P, 1024], F32, tag="psA")
                        for dp in range(DP):
                            for off in range(0, 1024, 512):
                                nc.tensor.matmul(
                                    pt[:, off:off + 512],
                                    lhsT=wk_f8[dp][:, :, c * P:(c + 1) * P],
                                    rhs=xp[dp][:, :, hf * 1024 + off:
                                               hf * 1024 + off + 512],
                                    start=(dp == 0), stop=(dp == DP - 1),
                                    perf_mode=DR)
                        _copy(rot2(c * 2 + hf),
                              kT_sb[c][:, hf * 1024:(hf + 1) * 1024], pt)
                # V (token-major) with ones column per head
                for t in range(NB):
                    pt = psA.tile([P, D], F32, tag="psA")
                    for dp in range(DP):
                        for off in range(0, D, 512):
                            nc.tensor.matmul(
                                pt[:, off:off + 512],
                                lhsT=xp[dp][:, :, t * P:(t + 1) * P],
                                rhs=wv_f8[dp][:, :, off:off + 512],
                                start=(dp == 0), stop=(dp == DP - 1),
                                perf_mode=DR)
                    vv = vp[t % QB]
                    _copy(rot2(t), vv[:, t // QB, :, 0:HD],
                          pt.rearrange("p (h e) -> p h e", h=H))
                    nc.gpsimd.memset(vv[:, t // QB, :, HD:HD + 1], 1.0)

            # phase-D weights/params + residual rows prefetch while
            # attention runs
            for cp in range(DP):
                nc.sync.dma_start(out=wo_f8[cp], in_=wall[3][cp])
            nc.gpsimd.dma_start(out=g_sb, in_=g_s)
            for j in range(QB):
                nc.sync.dma_start(out=xr_sb[j], in_=xres_s(j))

            # ---------------- Phase C: attention ----------------
            # Software-pipelined: step (h, jp) emits scores+exp; the
            # previous step's mask+AV (+ head normalize) trail one step
            # behind so the next scores are already in flight when the
            # Activation engine finishes the current exp.
            with tc.tile_pool(name="pexp", bufs=8) as pexp, \
                 tc.tile_pool(name="prec", bufs=2) as prec, \
                 tc.tile_pool(name="psS", bufs=3, space="PSUM") as psS, \
                 tc.tile_pool(name="psO", bufs=1, space="PSUM") as psO:
                po_by_h = {}
                pending = None

                def emit_tail(h, jp, et):
                    po = po_by_h[h]
                    p0 = jp * P
                    eng = nc.vector if (h * QB + jp) % 2 == 0 else nc.gpsimd
                    eng.tensor_mul(et[:, :, 0:P], et[:, :, 0:P], maskv)
                    lw = vp[jp][:, :, h, :]
                    for sa, se in _av_segments(p0, NQ):
                        # stop only on the terminal write of each PSUM
                        # bank (group tracking is per 2KB zero-region)
                        nc.tensor.matmul(
                            po[0:VW, sa:se],
                            lhsT=lw, rhs=et[:, :, sa - p0:se - p0],
                            start=(jp == 0),
                            stop=(jp % 4 == 3 and sa == p0),
                            perf_mode=DR)
                    if jp == QB - 1:
                        # normalize head h: broadcast 1/den across the 64
                        # hd partitions via a DRAM roundtrip
                        ch, r0 = h // 2, (h % 2) * HD
                        rec = prec.tile([1, NQ], F32, tag="rec", name="rec")
                        nc.vector.reciprocal(rec, po[HD:HD + 1, :])
                        rec_d = dpool.tile([NQ], F32, tag="rec_d",
                                           name="rec_d", bufs=2)
                        nc.sync.dma_start(out=rec_d, in_=rec)
                        rb = prec.tile([HD, NQ], F32, tag="rb", name="rb")
                        rb_bc = bass.AP(tensor=rec_d.tensor,
                                        offset=rec_d.offset,
                                        ap=[[0, HD], list(rec_d.ap[0])])
                        nc.sync.dma_start(out=rb, in_=rb_bc)
                        nc.vector.tensor_mul(
                            aT_f8[ch // 2][r0:r0 + HD, ch % 2, :],
                            po[0:HD, :], rb)
                        del po_by_h[h]

                for h in range(H):
                    ch, r0 = h // 2, (h % 2) * HD
                    po_by_h[h] = psO.tile([P, NQ], F32, tag="psO", name="po")
                    for jp in range(QB):
                        p0 = jp * P
                        ntail = NQ - p0
                        et = pexp.tile([P, 2 * NQ], F8, tag="expT", name="et") \
                            .rearrange("p (t q) -> p t q", t=2)
                        a = 0
                        while a < ntail:
                            w = min(512, ntail - a)
                            ps2 = psS.tile([P, 2 * 512], F32, tag="psS",
                                           name="ps2") \
                                .rearrange("p (t q) -> p t q", t=2)
                            for t in range(2):
                                kb = jp + QB * t
                                nc.tensor.matmul(
                                    ps2[:, t, 0:w],
                                    lhsT=kT_sb[ch][r0:r0 + HD,
                                                   kb * P:(kb + 1) * P],
                                    rhs=qT_sb[ch][r0:r0 + HD,
                                                  p0 + a:p0 + a + w],
                                    start=True, stop=True)
                            nc.scalar.activation(out=et[:, :, a:a + w],
                                                 in_=ps2[:, :, 0:w],
                                                 func=FP.Exp, scale=0.125)
                            a += w
                        if pending is not None:
                            emit_tail(*pending)
                        pending = (h, jp, et)
                if pending is not None:
                    emit_tail(*pending)

        # ---------------- Phase D: wo + residual + RMSNorm ----------------
        with tc.tile_pool(name="py", bufs=3) as pyp, \
             tc.tile_pool(name="psY", bufs=2, space="PSUM") as psY:
            for j in range(QB):
                py = psY.tile([P, D], F32, tag="psY")
                for cp in range(DP):
                    for off in range(0, D, 512):
                        nc.tensor.matmul(
                            py[:, off:off + 512],
                            lhsT=aT_f8[cp][:, :, j * P:(j + 1) * P],
                            rhs=wo_f8[cp][:, :, off:off + 512],
                            start=(cp == 0), stop=(cp == DP - 1),
                            perf_mode=DR)
                ysb = pyp.tile([P, D], F32, tag="ysb")
                nc.vector.tensor_add(ysb, py, xr_sb[j])
                sq = pyp.tile([P, D], F32, tag="sq")
                ss = pyp.tile([P, 1], F32, tag="ss")
                nc.scalar.activation(out=sq, in_=ysb, func=FP.Square,
                                     accum_out=ss)
                rstd = pyp.tile([P, 1], F32, tag="rstd")
                nc.scalar.activation(out=rstd, in_=ss, func=FP.Sqrt,
                                     scale=1.0 / D, bias=eps_sb)
                nc.vector.reciprocal(rstd, rstd)
                osb = pyp.tile([P, D], BF, tag="osb")
                nc.vector.scalar_tensor_tensor(
                    out=osb, in0=ysb, scalar=rstd, in1=g_sb,
                    op0=OP.mult, op1=OP.mult)
                nc.sync.dma_start(out=yout[j * P:(j + 1) * P, :], in_=osb)

    nc.compile()
    return nc


N_CORES = 8


def _make_runner(nc):
    import jax
    from jax.experimental.shard_map import shard_map
    from jax.sharding import Mesh, NamedSharding, PartitionSpec
    from concourse import bass2jax

    bass2jax.install_neuronx_cc_hook()
    partition_name = (nc.partition_id_tensor.name
                      if nc.partition_id_tensor else None)
    in_names, out_names, out_avals = [], [], []
    for alloc in nc.m.functions[0].allocations:
        if not isinstance(alloc, mybir.MemoryLocationSet):
            continue
        name = alloc.memorylocations[0].name
        if alloc.kind == "ExternalInput":
            if name != partition_name:
                in_names.append(name)
        elif alloc.kind == "ExternalOutput":
            out_names.append(name)
            out_avals.append(jax.core.ShapedArray(
                tuple(alloc.tensor_shape), mybir.dt.np(alloc.dtype)))
    n_params = len(in_names)
    # No output-seed operands: the kernel writes every element of y, so
    # the custom call's uninitialized result buffer is fully overwritten.
    all_in = list(in_names)
    if partition_name is not None:
        all_in.append(partition_name)

    def _body(*args):
        operands = list(args)
        if partition_name is not None:
            operands.append(bass2jax.partition_id_tensor())
        outs = bass2jax._bass_exec_p.bind(
            *operands,
            out_avals=tuple(out_avals),
            in_names=tuple(all_in),
            out_names=tuple(out_names),
            lowering_input_output_aliases=(),
            sim_require_finite=True,
            sim_require_nnan=True,
            nc=nc,
        )
        return tuple(outs)

    devices = jax.devices()[:N_CORES]
    mesh = Mesh(np.asarray(devices), ("core",))
    smapped = shard_map(_body, mesh=mesh,
                        in_specs=(PartitionSpec("core"),) * n_params,
                        out_specs=(PartitionSpec("core"),) * len(out_names),
                        check_rep=False)
    # Fast-dispatch (effect-free) compile keeps the per-launch client
    # overhead low so deep pipelines of in-flight executes stay fed.
    sh = NamedSharding(mesh, PartitionSpec("core"))
    in_sds = []
    for alloc in nc.m.functions[0].allocations:
        if not isinstance(alloc, mybir.MemoryLocationSet):
            continue
        name = alloc.memorylocations[0].name
        if alloc.kind == "ExternalInput" and name != partition_name:
            in_sds.append(jax.ShapeDtypeStruct(
                (N_CORES * alloc.tensor_shape[0], *alloc.tensor_shape[1:]),
                mybir.dt.np(alloc.dtype), sharding=sh))
    sharded = bass2jax.fast_dispatch_compile(
        lambda: jax.jit(smapped, keep_unused=True).lower(*in_sds).compile())
    return {"fn": sharded, "in_names": in_names, "out_names": out_names,
            "out_avals": out_avals, "mesh": mesh}


def _get_runner():
    if "runner" not in _NC_CACHE:
        if "nc" not in _NC_CACHE:
            _NC_CACHE["nc"] = build_nc()
        _NC_CACHE["runner"] = _make_runner(_NC_CACHE["nc"])
    return _NC_CACHE["runner"]


def _concat_inputs(r, in_maps):
    return [np.concatenate([np.asarray(in_maps[c][nm]) for c in range(N_CORES)],
                           axis=0)
            for nm in r["in_names"]]


def _run(in_maps):
    r = _get_runner()
    out_arrs = r["fn"](*_concat_inputs(r, in_maps))
    return [
        {nm: np.asarray(out_arrs[i]).reshape(N_CORES, *r["out_avals"][i].shape)[c]
         for i, nm in enumerate(r["out_names"])}
        for c in range(N_CORES)
    ]


def bench(in_maps, iters=3, depth=2048):
    """Per-launch steady-state time of the sharded NEFF execution.

    All inputs are device resident. Each rep launches `depth` executions
    back-to-back without blocking, then blocks once; the amortized
    total/depth is the per-launch service time with the axon-tunnel
    round-trip latency amortized away. Returns one amortized per-launch
    time (seconds) per rep.
    """
    import time
    import jax
    from jax.sharding import NamedSharding, PartitionSpec

    r = _get_runner()
    sh = NamedSharding(r["mesh"], PartitionSpec("core"))
    dev_in = [jax.device_put(a, sh) for a in _concat_inputs(r, in_maps)]
    jax.block_until_ready(dev_in)
    out = r["fn"](*dev_in)
    jax.block_until_ready(out)
    times = []
    for _ in range(iters):
        t0 = time.perf_counter()
        outs = [r["fn"](*dev_in) for _ in range(depth)]
        jax.block_until_ready(outs)
        times.append((time.perf_counter() - t0) / depth)
        del outs
    return times


def _rows(g):
    return np.arange(T).reshape(NB, P)[g::2].ravel()


def _pack_pairs(wT):
    """[D, D] f32 (rows = contraction dim) -> [DP, 128, 2, D] fp8 e4m3."""
    return np.ascontiguousarray(
        wT.reshape(DP, 2, P, D).transpose(0, 2, 1, 3)).astype(NPF8)


def kernel(**inputs):
    global LAST_RESULTS
    x = np.ascontiguousarray(np.asarray(inputs["x"], dtype=np.float32))
    wq = np.asarray(inputs["wq"], dtype=np.float32)
    wk = np.asarray(inputs["wk"], dtype=np.float32)
    wv = np.asarray(inputs["wv"], dtype=np.float32)
    wo = np.asarray(inputs["wo"], dtype=np.float32)
    g = np.ascontiguousarray(np.asarray(inputs["norm_g"], dtype=np.float32))

    if "nc" not in _NC_CACHE:
        _NC_CACHE["nc"] = build_nc()

    wallv = np.stack([_pack_pairs(wq.T), _pack_pairs(wk.T),
                      _pack_pairs(wv.T), _pack_pairs(wo.T)])
    # token perm puts the core's query blocks first, so slot 0 of every
    # key pair is the diagonal block (tri mask) and slot 1 is the other
    # block: fully masked for even groups, fully allowed for odd ones.
    tri = np.triu(np.ones((P, P), np.float32))  # allowed where k <= q
    masks = [np.stack([tri, np.zeros((P, P), np.float32)]),
             np.stack([tri, np.ones((P, P), np.float32)])]

    in_maps = []
    rows_g = [_rows(0), _rows(1)]
    for core in range(8):
        b, gidx = core // 2, core % 2
        rows = rows_g[gidx]
        perm = np.concatenate([rows, rows_g[1 - gidx]])
        xbig = np.concatenate([
            np.ascontiguousarray(x[b].T[:, perm]).ravel(),
            x[b][rows].ravel(),
            masks[gidx].ravel(),
            g,
        ])
        in_maps.append({"xbig": xbig, "wall": wallv})

    global LAST_IN_MAPS
    LAST_IN_MAPS = in_maps
    outs = _run(in_maps)

    y = np.empty((B, T, D), np.float32)
    for core in range(8):
        b, gidx = core // 2, core % 2
        y[b][rows_g[gidx]] = outs[core]["y"].astype(np.float32)
    return y


if __name__ == "__main__":
    rng = np.random.default_rng(0)
    ins = {
        "x": rng.standard_normal((B, T, D), dtype=np.float32),
        "wq": rng.standard_normal((D, D), dtype=np.float32) * 0.02,
        "wk": rng.standard_normal((D, D), dtype=np.float32) * 0.02,
        "wv": rng.standard_normal((D, D), dtype=np.float32) * 0.02,
        "wo": rng.standard_normal((D, D), dtype=np.float32) * 0.02,
        "norm_g": np.ones((D,), np.float32),
    }
    out = kernel(**ins)
    print("ok", out.shape, out.dtype)
